# revision 7
# baseline (speedup 1.0000x reference)
"""Trainium2 Bass kernel for nn_ClassifierModel (nms_detection).

Computation (reference):
    h    = relu(features @ conv_w + conv_b)        # (B,H,W,C)@(C,D) -> (B,H,W,D)
    flat = h.reshape(B, F)                         # F = H*W*D = 401408
    cls  = flat @ cls_w + cls_b                    # (B, 64)
    bbox = flat @ bbox_w + bbox_b                  # (B, 128)
    <tiny postprocessing with roi -> (B, P, 5)>

Only 128 of the 192 dense output columns can affect the result:
  * objectness is softmax over 2 logits -> only the logit DIFFERENCE matters,
    so cls_w's 64 columns fold into 32 difference columns on the host;
  * bbox regressor 0 (bbox cols 0..31) is dead: the reference overwrites
    x = roi0 - bb1*roi3, so bb0 never reaches the output.
The device therefore computes partial = flat @ W128 where
W128 = [cls_w[:,P:]-cls_w[:,:P] | bbox_w[:,32:]]  (F, 128).

Sharding: the flatten (contraction) dim F is split across the 8 cores by
slicing H into 8 chunks of 28 rows. Each core computes its conv slice and a
partial (B, 128) product against its F-slice of W128; the host sums the 8
partials and runs the tiny postprocessing. This reads each dense-weight
element exactly once across the machine (the weights dominate HBM traffic).

All matmul operands are fp8 (e4m3) to halve HBM traffic vs bf16; PSUM
accumulates fp32.  The big stage-2 GEMM uses DoubleRow perf mode (two fp8
k-groups per matmul, 0.5 cycles/row).  Subnormal-range inputs are prescaled
(conv_w x64, W128 x1024 - both have std far below e4m3's 2^-6 normal
threshold) and the host divides the partial sums by 64*1024.

Per-core device layout:
    featT (128,4,NB) : features slice ^T, (c%128, c//128, pix*B+b), fp8
    convw (128,4,256): conv_w*64, (c%128, c//128, d), fp8
    convb (128,2)    : conv_b*64 halves (d%128, d//128), fp32
    wmat  (128,NQ,2,128): W128*1024 rows, pair-tile j k-group g layout
                          [f%128, j, g, col] with f = j*256 + g*128 + (f%128)
    out   (16,128)   : partial fp32 sums for this core's F range

Stage 1 (per 448-column n-tile, per d-half q): PSUM[d',col] accumulates 4
k-tile matmuls, then ScalarE applies relu+bias writing fp8 into
hfull[:, q, cols] -- which is exactly the DoubleRow lhsT layout
([128, 2, 16]-slices) needed by stage 2, so no transposes anywhere.
Stage 2 streams wmat in chunks with a tapered tail so the final chunk's
exposed DMA+sem latency is small.
"""

import numpy as np

B = 16
H, W, C = 224, 7, 512
D = 256
P = 32
NCORES = 8
HSH = H // NCORES          # 28 rows of H per core
PIX = HSH * W              # 196 pixels per core per batch
FLOC = PIX * D             # 50176 contraction elements per core
NB = PIX * B               # 3136 stage-1 moving columns
NQ = PIX                   # 196 DoubleRow pair-tiles per core (256 f each)
NTILE = 448                # stage-1 moving tile (3136 = 7*448)
NTI = NB // NTILE          # 7 stage-1 n-tiles
CHUNKS = [24] * 7 + [12, 8, 5, 2, 1]   # W-stream pair-tile chunks (sum = 196)
NOUT = 128                 # device output columns: 32 cls-diff + 96 bbox
CONV_SCALE = 64.0          # conv_w prescale (std 0.02 -> e4m3 normal range)
W_SCALE = 1024.0           # W128 prescale  (std ~0.001 -> e4m3 normal range)
STRIDE = 16.0

_STATE = {}


def _build_module(reps=1):
    import concourse.mybir as mybir
    import concourse.tile as tile
    from concourse import bacc

    f32 = mybir.dt.float32
    fp8 = mybir.dt.float8e4
    nc = bacc.Bacc("TRN2", target_bir_lowering=False, debug=False)

    featT = nc.dram_tensor("featT", [128, 4, NB], fp8, kind="ExternalInput")
    convw = nc.dram_tensor("convw", [128, 4, D], fp8, kind="ExternalInput")
    convb = nc.dram_tensor("convb", [128, 2], f32, kind="ExternalInput")
    wmat = nc.dram_tensor("wmat", [128, NQ, 2, NOUT], fp8, kind="ExternalInput")
    if reps == 1:
        out = nc.dram_tensor("out", [16, NOUT], f32, kind="ExternalOutput")
    else:
        out = nc.dram_tensor("out", [reps, 16, NOUT], f32, kind="ExternalOutput")

    with tile.TileContext(nc) as tc:
        with (
            tc.tile_pool(name="res", bufs=2 if reps > 1 else 1) as res,
            tc.tile_pool(name="win", bufs=1) as win,
            tc.tile_pool(name="ps1", bufs=4, space="PSUM") as ps1p,
            tc.tile_pool(name="ps2", bufs=1, space="PSUM") as ps2p,
        ):
            for rep in range(reps):
                xt = res.tile([128, 4, NB], fp8, tag="xt", name="xt")
                nc.sync.dma_start(xt[:], featT[:])
                cw = res.tile([128, 4, D], fp8, tag="cw", name="cw")
                nc.sync.dma_start(cw[:], convw[:])
                cb = res.tile([128, 2], f32, tag="cb", name="cb")
                nc.sync.dma_start(cb[:], convb[:])
                hfull = res.tile([128, 2, NB], fp8, tag="hf", name="hfull")

                # All W chunks get dedicated SBUF tiles (~50KB/partition
                # total): no buffer-reuse waits can ever stall the DMA
                # stream behind PE progress.
                wcs = []
                pos = 0
                for ci, ch in enumerate(CHUNKS):
                    wc = win.tile([128, ch, 2, NOUT], fp8, tag=f"wc{ci}",
                                  name=f"wc{ci}")
                    nc.sync.dma_start(wc[:], wmat[:, pos:pos + ch])
                    wcs.append(wc)
                    pos += ch

                # Stage 1: hfull[:, q, n-tile] = relu(64*(conv_w[:,qhalf].T @
                # feat^T) + 64*b), written as fp8 in DoubleRow-lhsT layout.
                # DoubleRow over k-tile pairs halves PE time here too.
                for n in range(NTI):
                    for q in range(2):
                        ps = ps1p.tile([128, NTILE], f32, tag="ps",
                                       name=f"ps{n}_{q}")
                        for kk in range(2):
                            nc.tensor.matmul(
                                ps[:],
                                cw[:, 2 * kk:2 * kk + 2,
                                   q * 128:(q + 1) * 128],
                                xt[:, 2 * kk:2 * kk + 2,
                                   n * NTILE:(n + 1) * NTILE],
                                start=(kk == 0),
                                stop=(kk == 1),
                                perf_mode=mybir.MatmulPerfMode.DoubleRow,
                            )
                        nc.scalar.activation(
                            hfull[:, q, n * NTILE:(n + 1) * NTILE],
                            ps[:],
                            mybir.ActivationFunctionType.Relu,
                            bias=cb[:, q:q + 1],
                        )

                # Stage 2: acc(16,128) += DoubleRow(h-pair(128,2,16),
                # W-pair(128,2,128)) over 196 pair-tiles.
                acc = ps2p.tile([16, NOUT], f32, tag="acc", name="acc")
                pos = 0
                for ci, ch in enumerate(CHUNKS):
                    for t in range(ch):
                        j = pos + t
                        nc.tensor.matmul(
                            acc[:],
                            hfull[:, :, j * 16:(j + 1) * 16],
                            wcs[ci][:, t],
                            start=(j == 0),
                            stop=(j == NQ - 1),
                            perf_mode=mybir.MatmulPerfMode.DoubleRow,
                        )
                    pos += ch

                ot = res.tile([16, NOUT], f32, tag="ot", name="ot")
                nc.vector.tensor_copy(ot[:], acc[:])
                nc.sync.dma_start(out[:] if reps == 1 else out[rep], ot[:])

    nc.compile()
    return nc


def _prep_inputs(features, conv_w, conv_b, cls_w, bbox_w):
    import ml_dtypes

    f32 = np.float32
    e4 = ml_dtypes.float8_e4m3
    features = np.asarray(features, dtype=f32)
    conv_w = np.asarray(conv_w, dtype=f32)
    conv_b = np.asarray(conv_b, dtype=f32)
    cls_w = np.asarray(cls_w, dtype=f32)
    bbox_w = np.asarray(bbox_w, dtype=f32)

    convw_t = np.ascontiguousarray(
        (conv_w * CONV_SCALE).reshape(4, 128, D).transpose(1, 0, 2)).astype(e4)
    convb_t = np.ascontiguousarray(
        (conv_b * CONV_SCALE).reshape(2, 128).T).astype(f32)

    # W128 = [cls logit-difference (32) | live bbox cols 32..127 (96)]
    w128 = np.concatenate([cls_w[:, P:] - cls_w[:, :P], bbox_w[:, 32:]],
                          axis=1) * W_SCALE

    in_maps = []
    for i in range(NCORES):
        fi = features[:, i * HSH:(i + 1) * HSH, :, :].reshape(B, PIX, C)
        featT = np.ascontiguousarray(
            fi.transpose(2, 1, 0).reshape(4, 128, NB).transpose(1, 0, 2)
        ).astype(e4)

        # wmat[p, j, g, c] = W128[core_off + j*256 + g*128 + p, c]
        wl = np.ascontiguousarray(
            w128[i * FLOC:(i + 1) * FLOC]
            .reshape(NQ, 2, 128, NOUT).transpose(2, 0, 1, 3)).astype(e4)

        in_maps.append({
            "featT": featT,
            "convw": convw_t,
            "convb": convb_t,
            "wmat": wl,
        })
    return in_maps


def _run_device(in_maps, trace=False, **kw):
    from concourse.bass_utils import run_bass_kernel_spmd

    if "nc" not in _STATE:
        _STATE["nc"] = _build_module()
    nc = _STATE["nc"]
    return run_bass_kernel_spmd(
        nc, in_maps, core_ids=list(range(NCORES)), trace=trace, **kw
    )


def _postprocess(partial, roi, cls_b, bbox_b):
    f32 = np.float32
    partial = partial / f32(CONV_SCALE * W_SCALE)
    ld = partial[:, :32] + (cls_b[P:] - cls_b[:P]).astype(f32)
    obj = 1.0 / (1.0 + np.exp(-ld, dtype=f32))
    bbox96 = partial[:, 32:] + bbox_b[32:].astype(f32)
    bb = bbox96.reshape(B, 3, P)
    roi_img = roi.astype(f32) * f32(STRIDE)
    x = roi_img[:, :, 0] - bb[:, 0, :] * roi_img[:, :, 3]
    y = roi_img[:, :, 1]
    w = roi_img[:, :, 2] * np.exp(np.clip(bb[:, 1, :], -10.0, 10.0), dtype=f32)
    hh = roi_img[:, :, 3] * np.exp(np.clip(bb[:, 2, :], -10.0, 10.0), dtype=f32)
    return np.stack([x, y, w, hh, obj], axis=-1).astype(f32)


def kernel(features, roi, conv_w, conv_b, cls_w, cls_b, bbox_w, bbox_b):
    in_maps = _prep_inputs(features, conv_w, conv_b, cls_w, bbox_w)
    res = _run_device(in_maps)
    partial = np.zeros((B, NOUT), dtype=np.float64)
    for r in res.results:
        partial += np.asarray(r["out"], dtype=np.float64)
    return _postprocess(partial.astype(np.float32), np.asarray(roi),
                        np.asarray(cls_b), np.asarray(bbox_b))


# revision 8
# speedup vs baseline: 1.0022x; 1.0022x over previous
"""Trainium2 Bass kernel for nn_ClassifierModel (nms_detection).

Computation (reference):
    h    = relu(features @ conv_w + conv_b)        # (B,H,W,C)@(C,D) -> (B,H,W,D)
    flat = h.reshape(B, F)                         # F = H*W*D = 401408
    cls  = flat @ cls_w + cls_b                    # (B, 64)
    bbox = flat @ bbox_w + bbox_b                  # (B, 128)
    <tiny postprocessing with roi -> (B, P, 5)>

Only 128 of the 192 dense output columns can affect the result:
  * objectness is softmax over 2 logits -> only the logit DIFFERENCE matters,
    so cls_w's 64 columns fold into 32 difference columns on the host;
  * bbox regressor 0 (bbox cols 0..31) is dead: the reference overwrites
    x = roi0 - bb1*roi3, so bb0 never reaches the output.
The device therefore computes partial = flat @ W128 where
W128 = [cls_w[:,P:]-cls_w[:,:P] | bbox_w[:,32:]]  (F, 128).

Sharding: the flatten (contraction) dim F is split across the 8 cores by
slicing H into 8 chunks of 28 rows. Each core computes its conv slice and a
partial (B, 128) product against its F-slice of W128; the host sums the 8
partials and runs the tiny postprocessing. This reads each dense-weight
element exactly once across the machine (the weights dominate HBM traffic).

All matmul operands are fp8 (e4m3) to halve HBM traffic vs bf16; PSUM
accumulates fp32.  The big stage-2 GEMM uses DoubleRow perf mode (two fp8
k-groups per matmul, 0.5 cycles/row).  Subnormal-range inputs are prescaled
(conv_w x64, W128 x1024 - both have std far below e4m3's 2^-6 normal
threshold) and the host divides the partial sums by 64*1024.

Per-core device layout:
    featT (128,4,NB) : features slice ^T, (c%128, c//128, pix*B+b), fp8
    convw (128,4,256): conv_w*64, (c%128, c//128, d), fp8
    convb (128,2)    : conv_b*64 halves (d%128, d//128), fp32
    wmat  (128,NQ,2,128): W128*1024 rows, pair-tile j k-group g layout
                          [f%128, j, g, col] with f = j*256 + g*128 + (f%128)
    out   (16,128)   : partial fp32 sums for this core's F range

Stage 1 (per 448-column n-tile, per d-half q): PSUM[d',col] accumulates 4
k-tile matmuls, then ScalarE applies relu+bias writing fp8 into
hfull[:, q, cols] -- which is exactly the DoubleRow lhsT layout
([128, 2, 16]-slices) needed by stage 2, so no transposes anywhere.
Stage 2 streams wmat in chunks with a tapered tail so the final chunk's
exposed DMA+sem latency is small.
"""

import numpy as np

B = 16
H, W, C = 224, 7, 512
D = 256
P = 32
NCORES = 8
HSH = H // NCORES          # 28 rows of H per core
PIX = HSH * W              # 196 pixels per core per batch
FLOC = PIX * D             # 50176 contraction elements per core
NB = PIX * B               # 3136 stage-1 moving columns
NQ = PIX                   # 196 DoubleRow pair-tiles per core (256 f each)
NTILE = 448                # stage-1 moving tile (3136 = 7*448)
NTI = NB // NTILE          # 7 stage-1 n-tiles
# W-stream pair-tile chunks (sum = 196).  Tapered tail so each late chunk's
# 900ns completion-sem latency overlaps the next transfer; the final chunk is
# 2 pairs (512B/partition) -- the smallest size that avoids the <512B 2x DMA
# latency penalty.
CHUNKS = [24] * 7 + [12, 8, 4, 2, 2]
NOUT = 128                 # device output columns: 32 cls-diff + 96 bbox
CONV_SCALE = 64.0          # conv_w prescale (std 0.02 -> e4m3 normal range)
W_SCALE = 1024.0           # W128 prescale  (std ~0.001 -> e4m3 normal range)
STRIDE = 16.0

_STATE = {}


def _build_module(reps=1):
    import concourse.mybir as mybir
    import concourse.tile as tile
    from concourse import bacc

    f32 = mybir.dt.float32
    fp8 = mybir.dt.float8e4
    nc = bacc.Bacc("TRN2", target_bir_lowering=False, debug=False)

    featT = nc.dram_tensor("featT", [128, 4, NB], fp8, kind="ExternalInput")
    convw = nc.dram_tensor("convw", [128, 4, D], fp8, kind="ExternalInput")
    convb = nc.dram_tensor("convb", [128, 2], f32, kind="ExternalInput")
    wmat = nc.dram_tensor("wmat", [128, NQ, 2, NOUT], fp8, kind="ExternalInput")
    if reps == 1:
        out = nc.dram_tensor("out", [16, NOUT], f32, kind="ExternalOutput")
    else:
        out = nc.dram_tensor("out", [reps, 16, NOUT], f32, kind="ExternalOutput")

    with tile.TileContext(nc) as tc:
        with (
            tc.tile_pool(name="res", bufs=2 if reps > 1 else 1) as res,
            tc.tile_pool(name="win", bufs=1) as win,
            tc.tile_pool(name="ps1", bufs=4, space="PSUM") as ps1p,
            tc.tile_pool(name="ps2", bufs=1, space="PSUM") as ps2p,
        ):
            for rep in range(reps):
                xt = res.tile([128, 4, NB], fp8, tag="xt", name="xt")
                nc.sync.dma_start(xt[:], featT[:])
                cw = res.tile([128, 4, D], fp8, tag="cw", name="cw")
                nc.sync.dma_start(cw[:], convw[:])
                cb = res.tile([128, 2], f32, tag="cb", name="cb")
                nc.sync.dma_start(cb[:], convb[:])
                hfull = res.tile([128, 2, NB], fp8, tag="hf", name="hfull")

                # All W chunks get dedicated SBUF tiles (~50KB/partition
                # total): no buffer-reuse waits can ever stall the DMA
                # stream behind PE progress.
                wcs = []
                pos = 0
                for ci, ch in enumerate(CHUNKS):
                    wc = win.tile([128, ch, 2, NOUT], fp8, tag=f"wc{ci}",
                                  name=f"wc{ci}")
                    nc.sync.dma_start(wc[:], wmat[:, pos:pos + ch])
                    wcs.append(wc)
                    pos += ch

                # Stage 1: hfull[:, q, n-tile] = relu(64*(conv_w[:,qhalf].T @
                # feat^T) + 64*b), written as fp8 in DoubleRow-lhsT layout.
                # DoubleRow over k-tile pairs halves PE time here too.
                for n in range(NTI):
                    for q in range(2):
                        ps = ps1p.tile([128, NTILE], f32, tag="ps",
                                       name=f"ps{n}_{q}")
                        for kk in range(2):
                            nc.tensor.matmul(
                                ps[:],
                                cw[:, 2 * kk:2 * kk + 2,
                                   q * 128:(q + 1) * 128],
                                xt[:, 2 * kk:2 * kk + 2,
                                   n * NTILE:(n + 1) * NTILE],
                                start=(kk == 0),
                                stop=(kk == 1),
                                perf_mode=mybir.MatmulPerfMode.DoubleRow,
                            )
                        nc.scalar.activation(
                            hfull[:, q, n * NTILE:(n + 1) * NTILE],
                            ps[:],
                            mybir.ActivationFunctionType.Relu,
                            bias=cb[:, q:q + 1],
                        )

                # Stage 2: acc(16,128) += DoubleRow(h-pair(128,2,16),
                # W-pair(128,2,128)) over 196 pair-tiles.
                acc = ps2p.tile([16, NOUT], f32, tag="acc", name="acc")
                pos = 0
                for ci, ch in enumerate(CHUNKS):
                    for t in range(ch):
                        j = pos + t
                        nc.tensor.matmul(
                            acc[:],
                            hfull[:, :, j * 16:(j + 1) * 16],
                            wcs[ci][:, t],
                            start=(j == 0),
                            stop=(j == NQ - 1),
                            perf_mode=mybir.MatmulPerfMode.DoubleRow,
                        )
                    pos += ch

                ot = res.tile([16, NOUT], f32, tag="ot", name="ot")
                nc.vector.tensor_copy(ot[:], acc[:])
                nc.sync.dma_start(out[:] if reps == 1 else out[rep], ot[:])

    nc.compile()
    return nc


def _prep_inputs(features, conv_w, conv_b, cls_w, bbox_w):
    import ml_dtypes

    f32 = np.float32
    e4 = ml_dtypes.float8_e4m3
    features = np.asarray(features, dtype=f32)
    conv_w = np.asarray(conv_w, dtype=f32)
    conv_b = np.asarray(conv_b, dtype=f32)
    cls_w = np.asarray(cls_w, dtype=f32)
    bbox_w = np.asarray(bbox_w, dtype=f32)

    convw_t = np.ascontiguousarray(
        (conv_w * CONV_SCALE).reshape(4, 128, D).transpose(1, 0, 2)).astype(e4)
    convb_t = np.ascontiguousarray(
        (conv_b * CONV_SCALE).reshape(2, 128).T).astype(f32)

    # W128 = [cls logit-difference (32) | live bbox cols 32..127 (96)]
    w128 = np.concatenate([cls_w[:, P:] - cls_w[:, :P], bbox_w[:, 32:]],
                          axis=1) * W_SCALE

    in_maps = []
    for i in range(NCORES):
        fi = features[:, i * HSH:(i + 1) * HSH, :, :].reshape(B, PIX, C)
        featT = np.ascontiguousarray(
            fi.transpose(2, 1, 0).reshape(4, 128, NB).transpose(1, 0, 2)
        ).astype(e4)

        # wmat[p, j, g, c] = W128[core_off + j*256 + g*128 + p, c]
        wl = np.ascontiguousarray(
            w128[i * FLOC:(i + 1) * FLOC]
            .reshape(NQ, 2, 128, NOUT).transpose(2, 0, 1, 3)).astype(e4)

        in_maps.append({
            "featT": featT,
            "convw": convw_t,
            "convb": convb_t,
            "wmat": wl,
        })
    return in_maps


def _run_device(in_maps, trace=False, **kw):
    from concourse.bass_utils import run_bass_kernel_spmd

    if "nc" not in _STATE:
        _STATE["nc"] = _build_module()
    nc = _STATE["nc"]
    return run_bass_kernel_spmd(
        nc, in_maps, core_ids=list(range(NCORES)), trace=trace, **kw
    )


def _postprocess(partial, roi, cls_b, bbox_b):
    f32 = np.float32
    partial = partial / f32(CONV_SCALE * W_SCALE)
    ld = partial[:, :32] + (cls_b[P:] - cls_b[:P]).astype(f32)
    obj = 1.0 / (1.0 + np.exp(-ld, dtype=f32))
    bbox96 = partial[:, 32:] + bbox_b[32:].astype(f32)
    bb = bbox96.reshape(B, 3, P)
    roi_img = roi.astype(f32) * f32(STRIDE)
    x = roi_img[:, :, 0] - bb[:, 0, :] * roi_img[:, :, 3]
    y = roi_img[:, :, 1]
    w = roi_img[:, :, 2] * np.exp(np.clip(bb[:, 1, :], -10.0, 10.0), dtype=f32)
    hh = roi_img[:, :, 3] * np.exp(np.clip(bb[:, 2, :], -10.0, 10.0), dtype=f32)
    return np.stack([x, y, w, hh, obj], axis=-1).astype(f32)


def kernel(features, roi, conv_w, conv_b, cls_w, cls_b, bbox_w, bbox_b):
    in_maps = _prep_inputs(features, conv_w, conv_b, cls_w, bbox_w)
    res = _run_device(in_maps)
    partial = np.zeros((B, NOUT), dtype=np.float64)
    for r in res.results:
        partial += np.asarray(r["out"], dtype=np.float64)
    return _postprocess(partial.astype(np.float32), np.asarray(roi),
                        np.asarray(cls_b), np.asarray(bbox_b))
